# revision 20
# baseline (speedup 1.0000x reference)
"""Bass/Trainium2 kernel for CustomRNN (B=2048, T=512, I=1, H=64).

Math (per reference):
    xp[b,t,:] = x[b,t,0] * W_ih[:,0] + b_ih + b_hh
    h_{t+1}   = tanh(xp[:,t,:] + h_t @ W_hh.T),   h_0 = 0
    out       = h_T @ fc_w.T + fc_b              # [B, 1]

Sharding: ALL 2048 batch rows on ONE core. The axon-tunnel dispatch
overhead is per-device (~0.8 ms/device/call server-side; 8-core
shard_map measures 6-8 ms/call amortized vs ~1.0 ms single-device),
while the recurrence body is latency-bound, not throughput-bound: one
core runs the whole batch in ~554 us (ScalarE-saturated: per step two
tanh ACTs of [128, 512] ~= 1.1 us/step x 512 steps), which an 8-way
split would only cut to ~440 us/8-overlapped while adding ~7 ms of
dispatch. Weights are baked into the NEFF as consts.

Per-core design: the batch rows split into C independent "chains", each
a [S*64, F] tile (S batch halves stacked on the partition axis, F batch
columns; S*F batch rows per chain). Per chain per timestep:
  - mm_x: psum  = lhsT_x[r]^T @ xs_seg   (start=True)  -- the input term
    W_ih * x_t. lhsT_x[r] selects timestep r's row out of the statically
    staged x^T segment via a masked stationary (zeros except row r per
    64-row half). No data movement for x, ever; this matmul has no
    dependence on the recurrence so it runs off the critical path.
  - mm_h: psum += blockdiag(W_hh^T) @ h  (start=False) -- the recurrent
    term, the only op on the serial chain.
  - ACT (ScalarE): h' = tanh(psum + bias), bias = per-partition [S*64,1]
    copy of b_ih+b_hh, written straight where the next mm_h reads.
S=2 stacks two 64-row batch halves so one ACT covers 128 lanes,
halving the per-instruction bubble per batch element. Multiple chains
interleave on ScalarE so the mm/semaphore latency of one chain hides
under the other chains' ACTs.

The h tile hops to a fresh F-column segment every 64 steps (matmul
operand partition bases must stay 32-aligned, so a per-step sliding
window is illegal; in-place updates within a segment + the masked
stationary x selection need no per-step data movement at all).

fc tail: h_T lands in a [S*64, F] f32 tile; one matmul against a
block-column fc_w stationary gives [S, F], + fc_b via tensor_scalar.
"""

import sys

if "/opt/trn_rl_repo" not in sys.path:
    sys.path.insert(0, "/opt/trn_rl_repo")

import ml_dtypes
import numpy as np

B, T, I, H = 2048, 512, 1, 64
N_CORES = 1                    # knob: 1, 2, 4, or 8 (1 is fastest: the
                               # axon per-call dispatch overhead is
                               # per-device, and the recurrence is
                               # latency-bound, so one core with wider
                               # tiles wins)
N_SEG = T // H                 # 8 column segments

# chain layout name "c<n>s<S>": n chains, S batch-halves stacked on the
# partition axis; per-chain F = (B/N_CORES) / (n*S) batch columns.
CONFIG = "c2s2"
T_STEPS = T                    # override for timing experiments
K_REP = 1                      # in-NEFF repetitions (timing only)
N_SEG_IN = N_SEG               # staged-x segments (timing experiments)
PSUM_BUFS = None               # override PSUM pool depth (None = auto)

_CACHE = {}


def _chains():
    import re

    m = re.fullmatch(r"c(\d+)s(\d+)", CONFIG)
    n_ch, S = int(m.group(1)), int(m.group(2))
    b_core = B // N_CORES
    total = b_core // S
    assert total * S == b_core
    chains = []
    off = 0
    for i in range(n_ch):
        F = total // n_ch + (1 if i < total % n_ch else 0)
        assert F <= 512, "PSUM bank limit"
        chains.append((S, F, off))
        off += S * F
    assert off == b_core, (CONFIG, off)
    return chains


def _build(weights):
    from concourse import bacc, mybir, tile

    chains = _chains()
    # no collectives / no partition-id use: drop the implicit partition_id
    # ExternalInput so each execute ships one fewer buffer
    nc = bacc.Bacc(None, enable_partition_id=False)
    f32 = mybir.dt.float32
    bf16 = mybir.dt.bfloat16

    # single staged-x input: chain i's [S*H, N_SEG_IN*F] block lives at
    # column offset i*N_SEG_IN*F (chains share S, F in practice)
    xt_cols = [N_SEG_IN * F for (S, F, off) in chains]
    xt_ext = nc.dram_tensor(
        "xt", [chains[0][0] * H, sum(xt_cols)], bf16, kind="ExternalInput"
    )
    out_ext = nc.dram_tensor("out", [1, B // N_CORES], f32, kind="ExternalOutput")

    dram = {k: nc.inline_tensor(v, name=k) for k, v in weights.items()}

    from contextlib import ExitStack

    with tile.TileContext(nc) as tc:
        with ExitStack() as es:
            cpool = es.enter_context(tc.tile_pool(name="const", bufs=1))
            rpool = es.enter_context(tc.tile_pool(name="reg", bufs=1))
            fpool = es.enter_context(tc.tile_pool(name="fin", bufs=1))
            psum_bufs = PSUM_BUFS if PSUM_BUFS else (3 if len(chains) <= 2 else 2)
            pools = [
                es.enter_context(
                    tc.tile_pool(name=f"ps{i}", bufs=psum_bufs, space="PSUM")
                )
                for i in range(len(chains))
            ]

            sbuf = {}
            for k, t_dram in dram.items():
                tl = cpool.tile(list(t_dram.shape), t_dram.dtype, tag=k, name=f"sb_{k}")
                nc.sync.dma_start(out=tl[:], in_=t_dram[:])
                sbuf[k] = tl

            xs_all = rpool.tile(
                [chains[0][0] * H, sum(xt_cols)], bf16, tag="xs", name="xs"
            )
            nc.sync.dma_start(out=xs_all[:], in_=xt_ext[:])

            regions, xss, fins = [], [], []
            xt_off = 0
            for i, (S, F, off) in enumerate(chains):
                reg = rpool.tile([S * H, N_SEG * F], bf16, tag=f"reg{i}", name=f"reg{i}")
                nc.vector.memset(reg[:, 0:F], 0.0)
                regions.append(reg)
                xss.append(xt_off)  # base col of chain i in xs_all
                xt_off += N_SEG_IN * F
                fins.append(fpool.tile([S * H, F], f32, tag=f"fin{i}", name=f"fin{i}"))

            tanh = mybir.ActivationFunctionType.Tanh
            n_steps = T_STEPS
            for rep in range(K_REP):
                last_rep = rep == K_REP - 1
                for t in range(n_steps):
                    s, r = divmod(t, H)
                    s1 = ((t + 1) % n_steps) // H
                    # x-term matmuls first: no recurrence dependence, so
                    # they run early; the accumulation group closes on mm_h,
                    # the only op carrying the serial dependence.
                    pss = []
                    for i, (S, F, off) in enumerate(chains):
                        M = S * H
                        ps = pools[i].tile([M, F], f32, tag=f"ps{i}", name=f"ps{i}_{rep}_{t}")
                        pss.append(ps)
                        nc.tensor.matmul(
                            out=ps[:],
                            lhsT=sbuf[f"lx{S}"][:, r * M : (r + 1) * M],
                            rhs=xs_all[:, xss[i] + s * F : xss[i] + (s + 1) * F],
                            start=True,
                            stop=False,
                        )
                    for i, (S, F, off) in enumerate(chains):
                        ps = pss[i]
                        nc.tensor.matmul(
                            out=ps[:],
                            lhsT=sbuf[f"whh{S}"][:],
                            rhs=regions[i][:, s * F : (s + 1) * F],
                            start=False,
                            stop=True,
                        )
                        if t + 1 < n_steps and not (last_rep and t + 1 == n_steps):
                            dst = regions[i][:, s1 * F : (s1 + 1) * F]
                        elif not last_rep:
                            dst = regions[i][:, 0:F]
                        else:
                            dst = fins[i][:]
                        nc.scalar.activation(
                            dst, ps[:], tanh, bias=sbuf[f"bias{S}"][:]
                        )

            for i, (S, F, off) in enumerate(chains):
                pf = pools[i].tile([S, F], f32, tag=f"ps{i}", name=f"pf{i}")
                nc.tensor.matmul(
                    out=pf[:],
                    lhsT=sbuf["fcw"][0 : S * H, 0:S],
                    rhs=fins[i][:],
                    start=True,
                    stop=True,
                )
                ob = fpool.tile([S, F], f32, tag=f"ob{i}", name=f"ob{i}")
                nc.vector.tensor_scalar_add(ob[:], pf[:], sbuf["fcb"][0:S, 0:1])
                nc.sync.dma_start(
                    out=out_ext[0, off : off + S * F].rearrange(
                        "(p f) -> p f", p=S
                    ),
                    in_=ob[:],
                )

    nc.finalize()
    return nc


def _prep_weights(W_ih, W_hh, b_ih, b_hh, fc_w, fc_b):
    bf16 = ml_dtypes.bfloat16
    chains = _chains()
    svals = sorted({S for S, F, off in chains})
    w = {}
    wih = W_ih[:, 0]
    for S in svals:
        M = S * H
        whh = np.zeros((M, M), np.float32)
        for h in range(S):
            whh[h * H : (h + 1) * H, h * H : (h + 1) * H] = W_hh.T
        w[f"whh{S}"] = whh.astype(bf16)
        lx = np.zeros((M, H * M), np.float32)
        for r in range(H):
            for h in range(S):
                lx[h * H + r, r * M + h * H : r * M + (h + 1) * H] = wih
        w[f"lx{S}"] = lx.astype(bf16)
        w[f"bias{S}"] = np.tile(
            (b_ih + b_hh).astype(np.float32).reshape(H, 1), (S, 1)
        )
    fcw = np.zeros((2 * H, 2), np.float32)
    fcw[0:H, 0] = fc_w[0]
    fcw[H : 2 * H, 1] = fc_w[0]
    w["fcw"] = fcw
    w["fcb"] = np.full((2, 1), float(np.asarray(fc_b).reshape(-1)[0]), np.float32)
    return w


def _prep_x(x):
    """Per-core staged x^T per chain: xs[h*64+rho, s*F+j] =
    x[core_off + off + h*F + j, s*64 + rho]."""
    xf = np.ascontiguousarray(x.reshape(B, T))
    chains = _chains()
    b_core = B // N_CORES
    out = []
    for c in range(N_CORES):
        parts = []
        for i, (S, F, off) in enumerate(chains):
            xc = xf[c * b_core + off : c * b_core + off + S * F]  # [S*F, T]
            xc = xc[:, : N_SEG_IN * H]
            st = (
                xc.reshape(S, F, N_SEG_IN, H)
                .transpose(0, 3, 2, 1)
                .reshape(S * H, N_SEG_IN * F)
            )
            parts.append(st)
        out.append({"xt": np.concatenate(parts, 1).astype(ml_dtypes.bfloat16)})
    return out


def kernel(x, W_ih, W_hh, b_ih, b_hh, fc_w, fc_b):
    from concourse.bass_utils import run_bass_kernel_spmd

    x = np.asarray(x, np.float32)
    wargs = [
        np.asarray(a, np.float32)
        for a in (W_ih, W_hh, b_ih, b_hh, fc_w, fc_b)
    ]
    key = ("nc", CONFIG, N_CORES, *(a.tobytes() for a in wargs))
    if key not in _CACHE:
        _CACHE.clear()
        _CACHE[key] = _build(_prep_weights(*wargs))
    nc = _CACHE[key]

    in_maps = _prep_x(x)
    res = run_bass_kernel_spmd(nc, in_maps, list(range(N_CORES)))
    out = np.concatenate(
        [np.asarray(res.results[c]["out"][0], np.float32) for c in range(N_CORES)]
    )
    return out.reshape(B, 1)


if __name__ == "__main__":
    rng = np.random.default_rng(0)
    s = 1.0 / np.sqrt(H)
    inputs = {
        "x": rng.standard_normal((B, T, I)).astype(np.float32),
        "W_ih": rng.uniform(-s, s, (H, I)).astype(np.float32),
        "W_hh": rng.uniform(-s, s, (H, H)).astype(np.float32),
        "b_ih": rng.uniform(-s, s, H).astype(np.float32),
        "b_hh": rng.uniform(-s, s, H).astype(np.float32),
        "fc_w": rng.uniform(-s, s, (1, H)).astype(np.float32),
        "fc_b": rng.uniform(-s, s, 1).astype(np.float32),
    }
    out = kernel(**inputs)
    print("kernel out", out.shape, out[:4, 0])

